# revision 16
# baseline (speedup 1.0000x reference)
"""GCN layer (copy_src + segment_sum + concat + Linear) on 8 TRN2 NeuronCores.

Strategy (graph-parallel, dst-partitioned):
  - Nodes are partitioned across the 8 cores in contiguous ranges of R rows.
    Every core holds a full replica of the feature table (gather source)
    plus a transposed local slice for the self term.
  - Edges are routed on host to the core owning their dst, bucketed by src
    range (int16 index limit of dma_gather => buckets of 32768 source rows),
    and within each bucket sorted by dst window (128 dst rows); each
    (bucket, window) run is padded to a multiple of 128 edges.
  - On device, per chunk of edges: dma_gather (messages = feature[src]) into
    SBUF.  Per 128-edge group, a one-hot mask (is_equal against an iota tile)
    and a PE matmul perform the segment-sum:
        aggT[64 feat, 128 dst] += msg[128 edge, 64 feat].T @ mask[128 e, 128 d]
    accumulated in PSUM per run and drained into an SBUF aggT accumulator.
    No scatter DMA is used at all.
  - Final linear per window: outT = W1 @ featT_w + W2 @ aggT_w + b computed
    with two K=64 matmuls, then a PE transpose back to row-major.
"""

import os
import sys

for _p in ("/opt/trn_rl_repo",):
    if _p not in sys.path and os.path.isdir(_p):
        sys.path.insert(0, _p)

import numpy as np

import concourse.bass as bass
import concourse.mybir as mybir
import concourse.tile as tile
from concourse import bacc
from concourse.bass_utils import run_bass_kernel_spmd
from concourse.masks import make_identity

P = int(os.environ.get("GCN_CORES", "8"))  # cores
D = 64           # feature dim
TWO_D = 2 * D    # concat dim
BUCKET = 32768   # int16 index reach for dma_gather
CHUNK = 1024     # max edges per gather instruction (HW: >=2048 crashes)

F32 = mybir.dt.float32
I16 = mybir.dt.int16

LAST_EXEC_NS = None
LAST_RESULTS = None
LAST_WALL_S = None


def _round_up(x, m):
    return (x + m - 1) // m * m


def _wrap_idx(a):
    """[B] int16 -> [128, B/16]: index i at (i%16, i//16), tiled to 128 rows."""
    w = a.reshape(-1, 16).T  # [16, B/16]
    return np.tile(w, (8, 1))


def _prep(feature, src, dst, W, b):
    """Host-side sharding. Returns (meta, in_maps)."""
    N = feature.shape[0]
    R = _round_up((N + P - 1) // P, 128)   # rows per core
    NW = R // 128                          # dst windows per core
    n_buckets = (N + BUCKET - 1) // BUCKET

    feature = np.ascontiguousarray(feature, dtype=np.float32)
    src = np.asarray(src).astype(np.int64)
    dst = np.asarray(dst).astype(np.int64)

    part = dst // R
    order = np.argsort(part, kind="stable")
    src_s, dst_s = src[order], dst[order]
    counts_p = np.bincount(part, minlength=P)
    p_off = np.zeros(P + 1, np.int64)
    np.cumsum(counts_p, out=p_off[1:])

    # per (core, bucket): edges sorted by dst window, with run sizes per window
    per = [[None] * n_buckets for _ in range(P)]   # (src_loc, dstw, run_sizes[NW])
    for p in range(P):
        es = src_s[p_off[p]:p_off[p + 1]]
        ed = dst_s[p_off[p]:p_off[p + 1]] - p * R
        bkt = es // BUCKET
        bo = np.argsort(bkt, kind="stable")
        es, ed, bkt = es[bo], ed[bo], bkt[bo]
        cb = np.bincount(bkt, minlength=n_buckets)
        off = np.zeros(n_buckets + 1, np.int64)
        np.cumsum(cb, out=off[1:])
        for bu in range(n_buckets):
            sl = slice(off[bu], off[bu + 1])
            bes, bed = es[sl] - bu * BUCKET, ed[sl]
            o2 = np.argsort(bed // 128, kind="stable")
            bes, bed = bes[o2], bed[o2]
            runs = np.bincount(bed // 128, minlength=NW)
            per[p][bu] = (bes, bed, runs)

    # uniform run sizes: per (bucket, window) max over cores, rounded to 128
    RS = []  # RS[bu][w]
    for bu in range(n_buckets):
        sizes = []
        for w in range(NW):
            mx = max(int(per[p][bu][2][w]) for p in range(P))
            sizes.append(_round_up(mx, 128))
        RS.append(sizes)

    TG = sum(sum(s) for s in RS) // 128          # total 128-edge groups
    TC = sum(sum(s) for s in RS) // 16           # idx columns

    in_maps = []
    W1T = np.ascontiguousarray(W[:, :D].T, dtype=np.float32)   # [64 f, 64 o]
    W2T = np.ascontiguousarray(W[:, D:].T, dtype=np.float32)   # [64 f, 64 o]
    b64 = np.asarray(b, np.float32).reshape(D, 1)
    iotaT = np.tile(np.arange(128, dtype=np.float32), (128, 1))  # [e, i] = i
    for p in range(P):
        sc_cols, df_cols = [], []
        for bu in range(n_buckets):
            bes, bed, runs = per[p][bu]
            roff = np.zeros(NW + 1, np.int64)
            np.cumsum(runs, out=roff[1:])
            for w in range(NW):
                so = RS[bu][w]
                if so == 0:
                    continue
                ces = bes[roff[w]:roff[w + 1]]
                ced = bed[roff[w]:roff[w + 1]]
                pad = so - len(ces)
                es_p = np.concatenate([ces, np.zeros(pad, np.int64)]).astype(np.int16)
                dw_p = np.concatenate(
                    [ced - 128 * w, np.full(pad, -1.0)]).astype(np.float32)
                sc_cols.append(_wrap_idx(es_p))
                df_cols.append(dw_p.reshape(-1, 128).T)   # [128, so/128]
        featT = np.zeros((D, R), np.float32)
        lo, hi = p * R, min((p + 1) * R, N)
        featT[:, : hi - lo] = feature[lo:hi].T
        in_maps.append({
            "featD": feature,
            "featTL": featT,
            "srcI": np.ascontiguousarray(np.concatenate(sc_cols, axis=1)),
            "dstF": np.ascontiguousarray(np.concatenate(df_cols, axis=1)),
            "W1T": W1T,
            "W2T": W2T,
            "b64": b64,
            "iotaT": iotaT,
        })

    meta = dict(N=N, R=R, TG=TG, TC=TC,
                RS=tuple(tuple(s) for s in RS))
    return meta, in_maps


def _build(meta):
    N, R, TG, TC, RS = meta["N"], meta["R"], meta["TG"], meta["TC"], meta["RS"]
    NW = R // 128
    nc = bacc.Bacc("TRN2", target_bir_lowering=False, debug=False)

    featD = nc.dram_tensor("featD", [N, D], F32, kind="ExternalInput")
    featTL = nc.dram_tensor("featTL", [D, R], F32, kind="ExternalInput")
    srcI = nc.dram_tensor("srcI", [128, TC], I16, kind="ExternalInput")
    dstF = nc.dram_tensor("dstF", [128, TG], F32, kind="ExternalInput")
    W1Td = nc.dram_tensor("W1T", [D, D], F32, kind="ExternalInput")
    W2Td = nc.dram_tensor("W2T", [D, D], F32, kind="ExternalInput")
    b64d = nc.dram_tensor("b64", [D, 1], F32, kind="ExternalInput")
    iotaTd = nc.dram_tensor("iotaT", [128, 128], F32, kind="ExternalInput")
    outD = nc.dram_tensor("out", [R, D], F32, kind="ExternalOutput")

    with tile.TileContext(nc) as tc:
        with (
            tc.tile_pool(name="const", bufs=1) as cpool,
            tc.tile_pool(name="idx", bufs=4) as ipool,
            tc.tile_pool(name="msg", bufs=3) as mpool,
            tc.tile_pool(name="mask", bufs=6) as kpool,
            tc.tile_pool(name="osb", bufs=4) as opool,
            tc.tile_pool(name="ps_a", bufs=4, space="PSUM") as psa,
            tc.tile_pool(name="ps_o", bufs=2, space="PSUM") as pso,
        ):
            w1_sb = cpool.tile([D, D], F32)
            nc.sync.dma_start(w1_sb[:], W1Td[:])
            w2_sb = cpool.tile([D, D], F32)
            nc.sync.dma_start(w2_sb[:], W2Td[:])
            b_sb = cpool.tile([D, 1], F32)
            nc.sync.dma_start(b_sb[:], b64d[:])
            iota_sb = cpool.tile([128, 128], F32)
            nc.sync.dma_start(iota_sb[:], iotaTd[:])
            ident = cpool.tile([128, 128], F32)
            make_identity(nc, ident[:])
            featT_sb = cpool.tile([D, R], F32)
            nc.sync.dma_start(featT_sb[:], featTL[:])
            aggT_sb = cpool.tile([D, R], F32)
            nc.vector.memset(aggT_sb[:], 0.0)

            # Phase 1: gather + one-hot matmul segment-sum.
            # chunk plan per bucket: runs (w, ngroups) packed into <=CHUNK
            # gathers; run segments keep their own PSUM accumulation.
            col0 = 0   # idx column offset (16 edges per col)
            g0 = 0     # global group offset
            for bu, sizes in enumerate(RS):
                base = bu * BUCKET
                bsize = min(BUCKET, N - base)
                # chunks: list of (clen, [(w, gstart_in_chunk, ngroups)...])
                chunks, cur, cur_len = [], [], 0
                for w, so in enumerate(sizes):
                    rem = so
                    while rem > 0:
                        take = min(rem, CHUNK - cur_len)
                        cur.append((w, cur_len // 128, take // 128))
                        cur_len += take
                        rem -= take
                        if cur_len == CHUNK:
                            chunks.append((cur_len, cur))
                            cur, cur_len = [], 0
                if cur_len:
                    chunks.append((cur_len, cur))
                for clen, segs in chunks:
                    cols = clen // 16
                    sidx = ipool.tile([128, CHUNK // 16], I16, tag="sidx")
                    nc.sync.dma_start(sidx[:, :cols], srcI[:, col0:col0 + cols])
                    dstf = ipool.tile([128, CHUNK // 128], F32, tag="dstf")
                    ng = clen // 128
                    nc.sync.dma_start(dstf[:, :ng], dstF[:, g0:g0 + ng])
                    msg = mpool.tile([128, CHUNK // 128, D], F32, tag="msg")
                    nc.gpsimd.dma_gather(
                        msg[:, :ng, :],
                        featD[base:base + bsize, :],
                        sidx[:, :cols],
                        clen, clen, D,
                    )
                    for w, gs, ngr in segs:
                        ps = psa.tile([D, 128], F32)
                        for j in range(ngr):
                            g = gs + j
                            mask = kpool.tile([128, 128], F32, tag="mask")
                            nc.vector.tensor_tensor(
                                out=mask[:],
                                in0=dstf[:, g:g + 1].to_broadcast([128, 128]),
                                in1=iota_sb[:],
                                op=mybir.AluOpType.is_equal,
                            )
                            nc.tensor.matmul(
                                ps[:], lhsT=msg[:, g, :], rhs=mask[:],
                                start=(j == 0), stop=(j == ngr - 1),
                            )
                        wsl = slice(w * 128, (w + 1) * 128)
                        nc.vector.tensor_add(
                            aggT_sb[:, wsl], aggT_sb[:, wsl], ps[:])
                    col0 += cols
                    g0 += ng

            # Phase 2: outT_w = W1 @ featT_w + W2 @ aggT_w + b; transpose back.
            for w in range(NW):
                wsl = slice(w * 128, (w + 1) * 128)
                ot_ps = pso.tile([D, 128], F32, tag="ot")
                nc.tensor.matmul(ot_ps[:], lhsT=w1_sb[:], rhs=featT_sb[:, wsl],
                                 start=True, stop=False)
                nc.tensor.matmul(ot_ps[:], lhsT=w2_sb[:], rhs=aggT_sb[:, wsl],
                                 start=False, stop=True)
                ot_sb = opool.tile([D, 128], F32, tag="otsb")
                nc.vector.tensor_scalar_add(ot_sb[:], ot_ps[:], b_sb[:, :1])
                o_ps = pso.tile([128, D], F32, tag="ops")
                nc.tensor.matmul(o_ps[:], lhsT=ot_sb[:], rhs=ident[:D, :D],
                                 is_transpose=True)
                o_sb = opool.tile([128, D], F32, tag="osb")
                nc.scalar.copy(o_sb[:], o_ps[:])
                nc.sync.dma_start(outD[wsl, :], o_sb[:])

    nc.compile()
    return nc


_BUILD_CACHE = {}


def kernel(**inputs):
    global LAST_EXEC_NS, LAST_RESULTS
    feature = np.asarray(inputs["feature"])
    src = np.asarray(inputs["src"])
    dst = np.asarray(inputs["dst"])
    W = np.asarray(inputs["W"])
    b = np.asarray(inputs["b"])

    meta, in_maps = _prep(feature, src, dst, W, b)
    key = tuple(sorted((k, v) for k, v in meta.items()))
    if key not in _BUILD_CACHE:
        _BUILD_CACHE[key] = _build(meta)
    nc = _BUILD_CACHE[key]

    import time
    t0 = time.time()
    res = run_bass_kernel_spmd(nc, in_maps, list(range(P)))
    global LAST_WALL_S
    LAST_WALL_S = time.time() - t0
    LAST_EXEC_NS = res.exec_time_ns
    LAST_RESULTS = res
    N, R = meta["N"], meta["R"]
    out = np.concatenate([np.asarray(res.results[p]["out"]) for p in range(P)])
    return np.ascontiguousarray(out[:N])


# revision 17
# speedup vs baseline: 1.1194x; 1.1194x over previous
"""GCN layer (copy_src + segment_sum + concat + Linear) on 8 TRN2 NeuronCores.

Strategy (graph-parallel, dst-partitioned):
  - Nodes are partitioned across the 8 cores in contiguous ranges of R rows.
    Every core holds a full replica of the feature table (gather source)
    plus a transposed local slice for the self term.
  - Edges are routed on host to the core owning their dst, bucketed by src
    range (int16 index limit of dma_gather => buckets of 32768 source rows),
    and within each bucket sorted by dst window (128 dst rows); each
    (bucket, window) run is padded to a multiple of 128 edges.
  - On device, per chunk of edges: dma_gather (messages = feature[src]) into
    SBUF.  Per 128-edge group, a one-hot mask (is_equal against an iota tile)
    and a PE matmul perform the segment-sum:
        aggT[64 feat, 128 dst] += msg[128 edge, 64 feat].T @ mask[128 e, 128 d]
    accumulated in PSUM per run and drained into an SBUF aggT accumulator.
    No scatter DMA is used at all.
  - Final linear per window: outT = W1 @ featT_w + W2 @ aggT_w + b computed
    with two K=64 matmuls, then a PE transpose back to row-major.
"""

import os
import sys

for _p in ("/opt/trn_rl_repo",):
    if _p not in sys.path and os.path.isdir(_p):
        sys.path.insert(0, _p)

import numpy as np

import concourse.bass as bass
import concourse.mybir as mybir
import concourse.tile as tile
from concourse import bacc
from concourse.bass_utils import run_bass_kernel_spmd
from concourse.masks import make_identity

P = int(os.environ.get("GCN_CORES", "8"))  # cores
D = 64           # feature dim
TWO_D = 2 * D    # concat dim
BUCKET = 32768   # int16 index reach for dma_gather
CHUNK = 1024     # max edges per gather instruction (HW: >=2048 crashes)

F32 = mybir.dt.float32
I16 = mybir.dt.int16

LAST_EXEC_NS = None
LAST_RESULTS = None
LAST_WALL_S = None


def _round_up(x, m):
    return (x + m - 1) // m * m


def _wrap_idx(a):
    """[B] int16 -> [128, B/16]: index i at (i%16, i//16), tiled to 128 rows."""
    w = a.reshape(-1, 16).T  # [16, B/16]
    return np.tile(w, (8, 1))


def _prep(feature, src, dst, W, b):
    """Host-side sharding. Returns (meta, in_maps)."""
    N = feature.shape[0]
    R = _round_up((N + P - 1) // P, 128)   # rows per core
    NW = R // 128                          # dst windows per core
    n_buckets = (N + BUCKET - 1) // BUCKET

    feature = np.ascontiguousarray(feature, dtype=np.float32)
    src = np.asarray(src).astype(np.int64)
    dst = np.asarray(dst).astype(np.int64)

    part = dst // R
    order = np.argsort(part, kind="stable")
    src_s, dst_s = src[order], dst[order]
    counts_p = np.bincount(part, minlength=P)
    p_off = np.zeros(P + 1, np.int64)
    np.cumsum(counts_p, out=p_off[1:])

    # per (core, bucket): edges sorted by dst window, with run sizes per window
    per = [[None] * n_buckets for _ in range(P)]   # (src_loc, dstw, run_sizes[NW])
    for p in range(P):
        es = src_s[p_off[p]:p_off[p + 1]]
        ed = dst_s[p_off[p]:p_off[p + 1]] - p * R
        bkt = es // BUCKET
        bo = np.argsort(bkt, kind="stable")
        es, ed, bkt = es[bo], ed[bo], bkt[bo]
        cb = np.bincount(bkt, minlength=n_buckets)
        off = np.zeros(n_buckets + 1, np.int64)
        np.cumsum(cb, out=off[1:])
        for bu in range(n_buckets):
            sl = slice(off[bu], off[bu + 1])
            bes, bed = es[sl] - bu * BUCKET, ed[sl]
            o2 = np.argsort(bed // 128, kind="stable")
            bes, bed = bes[o2], bed[o2]
            runs = np.bincount(bed // 128, minlength=NW)
            per[p][bu] = (bes, bed, runs)

    # uniform run sizes: per (bucket, window) max over cores, rounded to 128
    RS = []  # RS[bu][w]
    for bu in range(n_buckets):
        sizes = []
        for w in range(NW):
            mx = max(int(per[p][bu][2][w]) for p in range(P))
            sizes.append(_round_up(mx, 128))
        RS.append(sizes)

    TG = sum(sum(s) for s in RS) // 128          # total 128-edge groups
    TC = sum(sum(s) for s in RS) // 16           # idx columns

    in_maps = []
    W1T = np.ascontiguousarray(W[:, :D].T, dtype=np.float32)   # [64 f, 64 o]
    W2T = np.ascontiguousarray(W[:, D:].T, dtype=np.float32)   # [64 f, 64 o]
    b64 = np.asarray(b, np.float32).reshape(D, 1)
    iotaT = np.tile(np.arange(128, dtype=np.float32), (128, 1))  # [e, i] = i
    for p in range(P):
        sc_cols, df_cols = [], []
        for bu in range(n_buckets):
            bes, bed, runs = per[p][bu]
            roff = np.zeros(NW + 1, np.int64)
            np.cumsum(runs, out=roff[1:])
            for w in range(NW):
                so = RS[bu][w]
                if so == 0:
                    continue
                ces = bes[roff[w]:roff[w + 1]]
                ced = bed[roff[w]:roff[w + 1]]
                pad = so - len(ces)
                es_p = np.concatenate([ces, np.zeros(pad, np.int64)]).astype(np.int16)
                dw_p = np.concatenate(
                    [ced - 128 * w, np.full(pad, -1.0)]).astype(np.float32)
                sc_cols.append(_wrap_idx(es_p))
                df_cols.append(dw_p.reshape(-1, 128).T)   # [128, so/128]
        featT = np.zeros((D, R), np.float32)
        lo, hi = p * R, min((p + 1) * R, N)
        featT[:, : hi - lo] = feature[lo:hi].T
        in_maps.append({
            "featD": feature,
            "featTL": featT,
            "srcI": np.ascontiguousarray(np.concatenate(sc_cols, axis=1)),
            "dstF": np.ascontiguousarray(np.concatenate(df_cols, axis=1)),
            "W1T": W1T,
            "W2T": W2T,
            "b64": b64,
            "iotaT": iotaT,
        })

    meta = dict(N=N, R=R, TG=TG, TC=TC,
                RS=tuple(tuple(s) for s in RS))
    return meta, in_maps


def _build(meta):
    N, R, TG, TC, RS = meta["N"], meta["R"], meta["TG"], meta["TC"], meta["RS"]
    NW = R // 128
    nc = bacc.Bacc("TRN2", target_bir_lowering=False, debug=False)

    featD = nc.dram_tensor("featD", [N, D], F32, kind="ExternalInput")
    featTL = nc.dram_tensor("featTL", [D, R], F32, kind="ExternalInput")
    srcI = nc.dram_tensor("srcI", [128, TC], I16, kind="ExternalInput")
    dstF = nc.dram_tensor("dstF", [128, TG], F32, kind="ExternalInput")
    W1Td = nc.dram_tensor("W1T", [D, D], F32, kind="ExternalInput")
    W2Td = nc.dram_tensor("W2T", [D, D], F32, kind="ExternalInput")
    b64d = nc.dram_tensor("b64", [D, 1], F32, kind="ExternalInput")
    iotaTd = nc.dram_tensor("iotaT", [128, 128], F32, kind="ExternalInput")
    outD = nc.dram_tensor("out", [R, D], F32, kind="ExternalOutput")

    with tile.TileContext(nc) as tc:
        with (
            tc.tile_pool(name="const", bufs=1) as cpool,
            tc.tile_pool(name="idx", bufs=4) as ipool,
            tc.tile_pool(name="msg", bufs=3) as mpool,
            tc.tile_pool(name="mask", bufs=6) as kpool,
            tc.tile_pool(name="osb", bufs=4) as opool,
            tc.tile_pool(name="ps_a", bufs=4, space="PSUM") as psa,
            tc.tile_pool(name="ps_o", bufs=2, space="PSUM") as pso,
        ):
            w1_sb = cpool.tile([D, D], F32)
            nc.sync.dma_start(w1_sb[:], W1Td[:])
            w2_sb = cpool.tile([D, D], F32)
            nc.sync.dma_start(w2_sb[:], W2Td[:])
            b_sb = cpool.tile([D, 1], F32)
            nc.sync.dma_start(b_sb[:], b64d[:])
            iota_sb = cpool.tile([128, 128], F32)
            nc.sync.dma_start(iota_sb[:], iotaTd[:])
            ident = cpool.tile([128, 128], F32)
            make_identity(nc, ident[:])
            featT_sb = cpool.tile([D, R], F32)
            nc.sync.dma_start(featT_sb[:], featTL[:])
            aggT_sb = cpool.tile([D, R], F32)
            nc.vector.memset(aggT_sb[:], 0.0)

            # Phase 1: gather + one-hot matmul segment-sum.
            # chunk plan per bucket: runs (w, ngroups) packed into <=CHUNK
            # gathers; run segments keep their own PSUM accumulation.
            col0 = 0   # idx column offset (16 edges per col)
            g0 = 0     # global group offset
            for bu, sizes in enumerate(RS):
                base = bu * BUCKET
                bsize = min(BUCKET, N - base)
                # chunks: list of (clen, [(w, gstart_in_chunk, ngroups)...])
                chunks, cur, cur_len = [], [], 0
                for w, so in enumerate(sizes):
                    rem = so
                    while rem > 0:
                        take = min(rem, CHUNK - cur_len)
                        cur.append((w, cur_len // 128, take // 128))
                        cur_len += take
                        rem -= take
                        if cur_len == CHUNK:
                            chunks.append((cur_len, cur))
                            cur, cur_len = [], 0
                if cur_len:
                    chunks.append((cur_len, cur))
                for clen, segs in chunks:
                    cols = clen // 16
                    sidx = ipool.tile([128, CHUNK // 16], I16, tag="sidx")
                    nc.sync.dma_start(sidx[:, :cols], srcI[:, col0:col0 + cols])
                    dstf = ipool.tile([128, CHUNK // 128], F32, tag="dstf")
                    ng = clen // 128
                    nc.sync.dma_start(dstf[:, :ng], dstF[:, g0:g0 + ng])
                    msg = mpool.tile([128, CHUNK // 128, D], F32, tag="msg")
                    nc.gpsimd.dma_gather(
                        msg[:, :ng, :],
                        featD[base:base + bsize, :],
                        sidx[:, :cols],
                        clen, clen, D,
                    )
                    for w, gs, ngr in segs:
                        ps = psa.tile([D, 128], F32)
                        # one batched one-hot build per segment: [128, G, 128]
                        mask = kpool.tile([128, CHUNK], F32, tag="mask")
                        nc.vector.tensor_tensor(
                            out=mask[:, : ngr * 128].rearrange(
                                "p (g i) -> p g i", i=128),
                            in0=dstf[:, gs:gs + ngr, None].to_broadcast(
                                [128, ngr, 128]),
                            in1=iota_sb[:][:, None, :].to_broadcast(
                                [128, ngr, 128]),
                            op=mybir.AluOpType.is_equal,
                        )
                        for j in range(ngr):
                            nc.tensor.matmul(
                                ps[:], lhsT=msg[:, gs + j, :],
                                rhs=mask[:, j * 128:(j + 1) * 128],
                                start=(j == 0), stop=(j == ngr - 1),
                            )
                        wsl = slice(w * 128, (w + 1) * 128)
                        nc.vector.tensor_add(
                            aggT_sb[:, wsl], aggT_sb[:, wsl], ps[:])
                    col0 += cols
                    g0 += ng

            # Phase 2: outT_w = W1 @ featT_w + W2 @ aggT_w + b; transpose back.
            for w in range(NW):
                wsl = slice(w * 128, (w + 1) * 128)
                ot_ps = pso.tile([D, 128], F32, tag="ot")
                nc.tensor.matmul(ot_ps[:], lhsT=w1_sb[:], rhs=featT_sb[:, wsl],
                                 start=True, stop=False)
                nc.tensor.matmul(ot_ps[:], lhsT=w2_sb[:], rhs=aggT_sb[:, wsl],
                                 start=False, stop=True)
                ot_sb = opool.tile([D, 128], F32, tag="otsb")
                nc.vector.tensor_scalar_add(ot_sb[:], ot_ps[:], b_sb[:, :1])
                o_ps = pso.tile([128, D], F32, tag="ops")
                nc.tensor.matmul(o_ps[:], lhsT=ot_sb[:], rhs=ident[:D, :D],
                                 is_transpose=True)
                o_sb = opool.tile([128, D], F32, tag="osb")
                nc.scalar.copy(o_sb[:], o_ps[:])
                nc.sync.dma_start(outD[wsl, :], o_sb[:])

    nc.compile()
    return nc


_BUILD_CACHE = {}


def kernel(**inputs):
    global LAST_EXEC_NS, LAST_RESULTS
    feature = np.asarray(inputs["feature"])
    src = np.asarray(inputs["src"])
    dst = np.asarray(inputs["dst"])
    W = np.asarray(inputs["W"])
    b = np.asarray(inputs["b"])

    meta, in_maps = _prep(feature, src, dst, W, b)
    key = tuple(sorted((k, v) for k, v in meta.items()))
    if key not in _BUILD_CACHE:
        _BUILD_CACHE[key] = _build(meta)
    nc = _BUILD_CACHE[key]

    import time
    t0 = time.time()
    res = run_bass_kernel_spmd(nc, in_maps, list(range(P)))
    global LAST_WALL_S
    LAST_WALL_S = time.time() - t0
    LAST_EXEC_NS = res.exec_time_ns
    LAST_RESULTS = res
    N, R = meta["N"], meta["R"]
    out = np.concatenate([np.asarray(res.results[p]["out"]) for p in range(P)])
    return np.ascontiguousarray(out[:N])
